# revision 4
# baseline (speedup 1.0000x reference)
"""Trainium2 Bass kernel for nn_MEGANCore (GATv2-style message-passing GNN).

Key insight: in the reference, _gatv2 gathers x_j = xp[col] and segment-sums
x_j * alpha by col; since softmax weights alpha sum to 1 within each segment
(and self-loops guarantee non-empty segments), the aggregation is exactly
xp = h @ W.  The whole network therefore collapses to a per-node linear
chain + layernorms + pooling:

    h_{l+1} = LN(h_l + (h_l@W0 + h_l@W1)/2) * scale_l + bias_l
    out = relu(segsum(h_L, batch) @ W1 + b1) @ W2 + b2

Further folding (bias_l == 0 in this problem; asserted):
  - residual+heads+centering+scale fold into one 64x64 matrix per layer:
        B_l = diag(scale_{l-1}) @ (I + (W0+W1)/2) @ (I - 11^T/64)
  - the LN row scalings commute through the chain; with eps=1e-5 dropped
    (error ~1e-5 relative, verified 3e-6 on the final output) the final
    per-node scale is just c4 = 1/sqrt(mean(h~_3^2)) where h~ is the
    unscaled chain  h~ = x @ B0 @ B1 @ B2 @ B3.
  - ln_scale[3] folds into W1.

Sharding: batch is sorted; 64 graphs -> 8 graphs per core, contiguous node
ranges, padded to NPAD.  Each core computes its 8 graph outputs end to end;
host concatenates.  Host prep = pure data layout (transpose/pad/one-hot).
"""

import numpy as np

HID = 64
NCORES = 8
GPC = 8                 # graphs per core
NBLK = 52               # 128-node blocks per core
NPAD = NBLK * 128       # 6656 padded nodes per core
CH = 512                # feat-major chunk width (psum bank)
NCH = NPAD // CH        # 13
QB = 13                 # L3 blocks per quarter
EPS_SQ = 1e-9           # guards rsqrt on zero-padded nodes

_prog = None


def _build_program():
    import concourse.tile as tile
    from concourse import bacc, mybir
    from contextlib import ExitStack

    f32 = mybir.dt.float32
    f32r = mybir.dt.float32r

    nc = bacc.Bacc(
        "TRN2", target_bir_lowering=False, debug=False, num_devices=NCORES
    )
    xT = nc.dram_tensor("xT", [64, NPAD], f32r, kind="ExternalInput").ap()
    Bc = nc.dram_tensor("Bc", [64, 256], f32r, kind="ExternalInput").ap()
    Mp = nc.dram_tensor("Mp", [128, NBLK * GPC], f32, kind="ExternalInput").ap()
    W1 = nc.dram_tensor("W1", [64, 32], f32r, kind="ExternalInput").ap()
    b1 = nc.dram_tensor("b1", [32, 1], f32, kind="ExternalInput").ap()
    W2 = nc.dram_tensor("W2", [32, 1], f32r, kind="ExternalInput").ap()
    b2 = nc.dram_tensor("b2", [1, 1], f32, kind="ExternalInput").ap()
    ey = nc.dram_tensor("ey", [8, 8], f32, kind="ExternalInput").ap()
    out = nc.dram_tensor("out", [1, GPC], f32, kind="ExternalOutput").ap()

    with tile.TileContext(nc) as tc:
        with ExitStack() as ctx:
            _body(ctx, tc, nc, mybir, xT, Bc, Mp, W1, b1, W2, b2, ey, out)
    nc.compile()
    return nc


def _body(ctx, tc, nc, mybir, xT, Bc, Mp, W1, b1, W2, b2, ey, out):
    f32 = mybir.dt.float32
    f32r = mybir.dt.float32r
    AF = mybir.ActivationFunctionType
    AX = mybir.AxisListType
    ALU = mybir.AluOpType

    const = ctx.enter_context(tc.tile_pool(name="const", bufs=1))
    hpool = ctx.enter_context(tc.tile_pool(name="hbuf", bufs=2))
    spool = ctx.enter_context(tc.tile_pool(name="scr", bufs=1))
    cps = ctx.enter_context(tc.tile_pool(name="cps", bufs=2, space="PSUM"))
    l3p = ctx.enter_context(tc.tile_pool(name="l3p", bufs=2, space="PSUM"))
    gps = ctx.enter_context(tc.tile_pool(name="gps", bufs=1, space="PSUM"))

    Bsb = const.tile([64, 256], f32r, tag="Bsb")
    nc.sync.dma_start(Bsb[:], Bc)
    Mpsb = const.tile([128, NBLK * GPC], f32, tag="Mpsb")
    nc.sync.dma_start(Mpsb[:], Mp)
    W1sb = const.tile([64, 32], f32r, tag="W1sb")
    nc.sync.dma_start(W1sb[:], W1)
    b1sb = const.tile([32, 1], f32, tag="b1sb")
    nc.sync.dma_start(b1sb[:], b1)
    W2sb = const.tile([32, 1], f32r, tag="W2sb")
    nc.sync.dma_start(W2sb[:], W2)
    b2sb = const.tile([1, 1], f32, tag="b2sb")
    nc.sync.dma_start(b2sb[:], b2)
    eysb = const.tile([8, 8], f32, tag="eysb")
    nc.sync.dma_start(eysb[:], ey)

    # ---- load x (feat-major, host-transposed) ----
    h = hpool.tile([64, NPAD], f32r, tag="h")
    for j in range(NCH):
        nc.sync.dma_start(h[:, j * CH:(j + 1) * CH], xT[:, j * CH:(j + 1) * CH])

    # ---- layers 0..2: feat-major chain  hT_{l+1} = B_l^T @ hT_l ----
    for l in range(3):
        hn = hpool.tile([64, NPAD], f32r, tag="h")
        for j in range(NCH):
            ps = cps.tile([64, CH], f32, tag="cp")
            nc.tensor.matmul(
                ps[:],
                Bsb[:, l * 64:(l + 1) * 64],
                h[:, j * CH:(j + 1) * CH],
                start=True, stop=True,
            )
            if j % 2 == 0:
                nc.vector.tensor_copy(hn[:, j * CH:(j + 1) * CH], ps[:])
            else:
                nc.scalar.copy(hn[:, j * CH:(j + 1) * CH], ps[:])
        h = hn

    # ---- layer 3: flipped to node-major; also second moment per node ----
    y3 = spool.tile([128, NBLK * 64], f32r, tag="y3")
    sq = spool.tile([128, NBLK * 64], f32, tag="sq")
    msq = spool.tile([128, NBLK], f32, tag="msq")
    for q in range(4):
        ps = l3p.tile([128, QB * 64], f32, tag="l3")
        for i in range(QB):
            t = q * QB + i
            nc.tensor.matmul(
                ps[:, i * 64:(i + 1) * 64],
                h[:, t * 128:(t + 1) * 128],
                Bsb[:, 192:256],
                start=True, stop=True,
            )
        nc.vector.tensor_copy(y3[:, q * QB * 64:(q + 1) * QB * 64], ps[:])
        nc.scalar.square(sq[:, q * QB * 64:(q + 1) * QB * 64], ps[:])
        nc.vector.tensor_reduce(
            msq[:, q * QB:(q + 1) * QB],
            sq[:, q * QB * 64:(q + 1) * QB * 64].rearrange(
                "p (b f) -> p b f", f=64
            ),
            axis=AX.X, op=ALU.add,
        )

    # ---- c4 = 1/sqrt(msq/64 + eps), fold into pooling weights ----
    epsb = const.tile([128, 1], f32, tag="epsb")
    nc.vector.memset(epsb[:], EPS_SQ)
    c4a = spool.tile([128, NBLK], f32, tag="c4a")
    nc.scalar.activation(c4a[:], msq[:], AF.Sqrt, bias=epsb[:], scale=1.0 / 64)
    c4 = spool.tile([128, NBLK], f32, tag="c4")
    nc.vector.reciprocal(c4[:], c4a[:])

    mp2 = spool.tile([128, NBLK * GPC], f32r, tag="mp2")
    for t in range(NBLK):
        nc.vector.tensor_scalar_mul(
            mp2[:, t * GPC:(t + 1) * GPC],
            Mpsb[:, t * GPC:(t + 1) * GPC],
            c4[:, t:t + 1],
        )

    # ---- pooling: g[8,64] = sum_t Mpool'[:,t]^T @ y3[:,t] ----
    g = gps.tile([8, 64], f32, tag="gmlp")
    for t in range(NBLK):
        nc.tensor.matmul(
            g[:],
            mp2[:, t * GPC:(t + 1) * GPC],
            y3[:, t * 64:(t + 1) * 64],
            start=(t == 0), stop=(t == NBLK - 1),
        )

    # ---- MLP head ----
    gsb = spool.tile([8, 64], f32, tag="gsb")
    nc.vector.tensor_copy(gsb[:], g[:])
    gT = gps.tile([64, 8], f32, tag="gmlp")
    nc.tensor.transpose(gT[:], gsb[:], eysb[:])
    gTsb = spool.tile([64, 8], f32r, tag="gTsb")
    nc.vector.tensor_copy(gTsb[:], gT[:])
    hid = gps.tile([32, 8], f32, tag="gmlp")
    nc.tensor.matmul(
        hid[:], W1sb[:], gTsb[:],
        start=True, stop=True,
    )
    hsb = spool.tile([32, 8], f32r, tag="hsb")
    nc.scalar.activation(hsb[:], hid[:], AF.Relu, bias=b1sb[:, 0:1], scale=1.0)
    o = gps.tile([1, 8], f32, tag="gmlp")
    nc.tensor.matmul(
        o[:], W2sb[:], hsb[:],
        start=True, stop=True,
    )
    osb = spool.tile([1, 8], f32, tag="osb")
    nc.scalar.activation(osb[:], o[:], AF.Identity, bias=b2sb[:, 0:1], scale=1.0)
    nc.sync.dma_start(out, osb[:])


def _prep_inputs(inputs):
    x = np.ascontiguousarray(np.asarray(inputs["x"], dtype=np.float32))
    batch = np.asarray(inputs["batch"]).astype(np.int64)
    Wn = np.asarray(inputs["Wn"], dtype=np.float32)
    ln_scale = np.asarray(inputs["ln_scale"], dtype=np.float32)
    ln_bias = np.asarray(inputs["ln_bias"], dtype=np.float32)
    W1 = np.asarray(inputs["W1"], dtype=np.float32)
    b1 = np.asarray(inputs["b1"], dtype=np.float32)
    W2 = np.asarray(inputs["W2"], dtype=np.float32)
    b2 = np.asarray(inputs["b2"], dtype=np.float32)
    assert np.allclose(ln_bias, 0.0), "kernel assumes ln_bias == 0"

    C = (np.eye(HID) - np.ones((HID, HID)) / HID).astype(np.float32)
    Bl = []
    for l in range(4):
        A = np.eye(HID, dtype=np.float32) + (Wn[l, 0] + Wn[l, 1]) * 0.5
        S = (
            np.diag(ln_scale[l - 1]).astype(np.float32)
            if l > 0 else np.eye(HID, dtype=np.float32)
        )
        Bl.append(np.ascontiguousarray((S @ A @ C).astype(np.float32)))
    Bcat = np.ascontiguousarray(np.concatenate(Bl, axis=1))
    W1p = np.ascontiguousarray(
        (np.diag(ln_scale[3]).astype(np.float32) @ W1).astype(np.float32)
    )

    bounds = np.searchsorted(batch, np.arange(0, 65, GPC))
    in_maps = []
    for c in range(NCORES):
        s, e = int(bounds[c]), int(bounds[c + 1])
        n = e - s
        assert n <= NPAD, f"core {c} shard {n} > NPAD {NPAD}"
        xTc = np.zeros((64, NPAD), dtype=np.float32)
        xTc[:, :n] = x[s:e].T
        mp = np.zeros((128, NBLK * GPC), dtype=np.float32)
        gb = (batch[s:e] - GPC * c).astype(np.int64)
        idx = np.arange(n)
        mp[idx % 128, (idx // 128) * GPC + gb] = 1.0
        in_maps.append(
            dict(
                xT=xTc,
                Bc=Bcat,
                Mp=np.ascontiguousarray(mp),
                W1=W1p,
                b1=np.ascontiguousarray(b1.reshape(32, 1)),
                W2=np.ascontiguousarray(W2.reshape(32, 1)),
                b2=np.ascontiguousarray(b2.reshape(1, 1)),
                ey=np.eye(8, dtype=np.float32),
            )
        )
    return in_maps


def kernel(**inputs):
    global _prog
    from concourse import bass_utils

    in_maps = _prep_inputs(inputs)
    if _prog is None:
        _prog = _build_program()
    res = bass_utils.run_bass_kernel_spmd(
        _prog, in_maps, core_ids=list(range(NCORES))
    )
    outs = [np.asarray(res.results[c]["out"]).reshape(GPC) for c in range(NCORES)]
    return np.concatenate(outs).reshape(64, 1).astype(np.float32)


# revision 7
# speedup vs baseline: 9017.6501x; 9017.6501x over previous
"""Trainium2 Bass kernel for nn_MEGANCore (GATv2-style message-passing GNN).

Key insight 1: in the reference, _gatv2 gathers x_j = xp[col] and segment-sums
x_j * alpha by col; softmax weights alpha sum to 1 within each segment (and
self-loops guarantee non-empty segments), so the aggregation is exactly
xp = h @ W: the edges never matter.  The network collapses to a per-node
linear chain + layernorms + pooling + MLP.

Key insight 2 (folding): with ln_bias == 0 (asserted), each layer is
    h_{l+1} = rstd_l * (h_l @ B_l),   B_l = diag(scale_{l-1}) (I + (W0+W1)/2) C
with C = I - 11^T/64 the centering matrix and rstd a per-node scalar.
Per-node scalars commute through the chain; dropping the LN eps=1e-5 inside
the chain (verified 3e-6 absmax-relative on the final output) the scalars
all cancel except a final c4 = 1/sqrt(mean((x @ B*)^2)) with
B* = B0@B1@B2@B3 precomputed on host.  The device computes only:

    h~ = x @ B*                    (one 64x64 matmul per 128-node block)
    c4 = rsqrt(mean(h~^2, feat))   (per node)
    g  = (Mpool * c4)^T @ h~       (pooling, 8 graphs/core)
    out = relu(g@W1'+b1)@W2+b2     (W1' = diag(ln_scale[3]) @ W1)

Sharding: batch is sorted; 64 graphs -> 8 graphs per core, contiguous node
ranges padded to NPAD.  Host prep is pure data layout (transpose/pad/
one-hot/weight folding).  Matmuls run as float32r (full fp32 storage,
fast PE mode; measured 2.3e-4 absmax-relative error), fp32 statistics.
"""

import numpy as np

HID = 64
NCORES = 8
GPC = 8                 # graphs per core
NBLK = 52               # 128-node blocks per core
NPAD = NBLK * 128       # 6656 padded nodes per core
QB = 13                 # blocks per psum quarter
EPS_SQ = 1e-9           # guards rsqrt on zero-padded nodes

_prog = None


def _build_program():
    import concourse.tile as tile
    from concourse import bacc, mybir
    from contextlib import ExitStack

    f32 = mybir.dt.float32
    f32r = mybir.dt.float32r
    bf16 = mybir.dt.bfloat16

    nc = bacc.Bacc(
        "TRN2", target_bir_lowering=False, debug=False, num_devices=NCORES
    )
    xT = nc.dram_tensor("xT", [64, NPAD], f32r, kind="ExternalInput").ap()
    Bs = nc.dram_tensor("Bs", [64, 64], f32r, kind="ExternalInput").ap()
    Mp = nc.dram_tensor("Mp", [128, NBLK * GPC], f32, kind="ExternalInput").ap()
    W1 = nc.dram_tensor("W1", [64, 32], f32r, kind="ExternalInput").ap()
    b1 = nc.dram_tensor("b1", [32, 1], f32, kind="ExternalInput").ap()
    W2 = nc.dram_tensor("W2", [32, 1], f32r, kind="ExternalInput").ap()
    b2 = nc.dram_tensor("b2", [1, 1], f32, kind="ExternalInput").ap()
    ey = nc.dram_tensor("ey", [8, 8], f32, kind="ExternalInput").ap()
    out = nc.dram_tensor("out", [1, GPC], f32, kind="ExternalOutput").ap()

    with tile.TileContext(nc) as tc:
        with ExitStack() as ctx:
            _body(ctx, tc, nc, mybir, xT, Bs, Mp, W1, b1, W2, b2, ey, out)
    nc.compile()
    return nc


def _body(ctx, tc, nc, mybir, xT, Bs, Mp, W1, b1, W2, b2, ey, out):
    f32 = mybir.dt.float32
    f32r = mybir.dt.float32r
    bf16 = mybir.dt.bfloat16
    AF = mybir.ActivationFunctionType
    AX = mybir.AxisListType
    ALU = mybir.AluOpType

    const = ctx.enter_context(tc.tile_pool(name="const", bufs=1))
    spool = ctx.enter_context(tc.tile_pool(name="scr", bufs=1))
    xpool = ctx.enter_context(tc.tile_pool(name="xp", bufs=1))
    l3p = ctx.enter_context(tc.tile_pool(name="l3p", bufs=2, space="PSUM"))
    gps = ctx.enter_context(tc.tile_pool(name="gps", bufs=1, space="PSUM"))

    Bsb = const.tile([64, 64], f32r, tag="Bsb")
    nc.sync.dma_start(Bsb[:], Bs)
    Mpsb = const.tile([128, NBLK * GPC], f32, tag="Mpsb")
    nc.sync.dma_start(Mpsb[:], Mp)
    W1sb = const.tile([64, 32], f32r, tag="W1sb")
    nc.sync.dma_start(W1sb[:], W1)
    b1sb = const.tile([32, 1], f32, tag="b1sb")
    nc.sync.dma_start(b1sb[:], b1)
    W2sb = const.tile([32, 1], f32r, tag="W2sb")
    nc.sync.dma_start(W2sb[:], W2)
    b2sb = const.tile([1, 1], f32, tag="b2sb")
    nc.sync.dma_start(b2sb[:], b2)
    eysb = const.tile([8, 8], f32, tag="eysb")
    nc.sync.dma_start(eysb[:], ey)
    epsb = const.tile([128, 1], f32, tag="epsb")
    nc.vector.memset(epsb[:], EPS_SQ)

    # ---- load x (feat-major, host-transposed), per-quarter chunks ----
    xsb = xpool.tile([64, NPAD], f32r, tag="xsb")
    for q in range(4):
        nc.sync.dma_start(
            xsb[:, q * QB * 128:(q + 1) * QB * 128],
            xT[:, q * QB * 128:(q + 1) * QB * 128],
        )

    # ---- h~ = x @ B* per 128-node block (node-major out), stats, evict ----
    y3 = spool.tile([128, NBLK * 64], f32r, tag="y3")
    sq = spool.tile([128, NBLK * 64], f32, tag="sq")
    msq = spool.tile([128, NBLK], f32, tag="msq")
    for q in range(4):
        ps = l3p.tile([128, QB * 64], f32, tag="l3")
        for i in range(QB):
            t = q * QB + i
            nc.tensor.matmul(
                ps[:, i * 64:(i + 1) * 64],
                xsb[:, t * 128:(t + 1) * 128],
                Bsb[:],
                start=True, stop=True,
            )
        half = QB * 64 // 2  # split eviction DVE/ACT
        q0 = q * QB * 64
        nc.vector.tensor_copy(y3[:, q0:q0 + half], ps[:, :half])
        nc.scalar.copy(y3[:, q0 + half:q0 + QB * 64], ps[:, half:])
        nc.scalar.square(sq[:, q0:q0 + QB * 64], ps[:])
        nc.vector.tensor_reduce(
            msq[:, q * QB:(q + 1) * QB],
            sq[:, q0:q0 + QB * 64].rearrange("p (b f) -> p b f", f=64),
            axis=AX.X, op=ALU.add,
        )

    # ---- c4 = 1/sqrt(msq/64 + eps), folded into pooling weights ----
    c4a = spool.tile([128, NBLK], f32, tag="c4a")
    nc.scalar.activation(c4a[:], msq[:], AF.Sqrt, bias=epsb[:], scale=1.0 / 64)
    c4 = spool.tile([128, NBLK], f32, tag="c4")
    nc.vector.reciprocal(c4[:], c4a[:])

    mp2 = spool.tile([128, NBLK * GPC], f32r, tag="mp2")
    for t in range(NBLK):
        nc.vector.tensor_scalar_mul(
            mp2[:, t * GPC:(t + 1) * GPC],
            Mpsb[:, t * GPC:(t + 1) * GPC],
            c4[:, t:t + 1],
        )

    # ---- pooling: g[8,64] = sum_t (Mpool*c4)[:,t]^T @ y3[:,t] ----
    g = gps.tile([8, 64], f32, tag="gmlp")
    for t in range(NBLK):
        nc.tensor.matmul(
            g[:],
            mp2[:, t * GPC:(t + 1) * GPC],
            y3[:, t * 64:(t + 1) * 64],
            start=(t == 0), stop=(t == NBLK - 1),
        )

    # ---- MLP head ----
    gsb = spool.tile([8, 64], f32, tag="gsb")
    nc.vector.tensor_copy(gsb[:], g[:])
    gT = gps.tile([64, 8], f32, tag="gmlp")
    nc.tensor.transpose(gT[:], gsb[:], eysb[:])
    gTsb = spool.tile([64, 8], f32r, tag="gTsb")
    nc.vector.tensor_copy(gTsb[:], gT[:])
    hid = gps.tile([32, 8], f32, tag="gmlp")
    nc.tensor.matmul(hid[:], W1sb[:], gTsb[:], start=True, stop=True)
    hsb = spool.tile([32, 8], f32r, tag="hsb")
    nc.scalar.activation(hsb[:], hid[:], AF.Relu, bias=b1sb[:, 0:1], scale=1.0)
    o = gps.tile([1, 8], f32, tag="gmlp")
    nc.tensor.matmul(o[:], W2sb[:], hsb[:], start=True, stop=True)
    osb = spool.tile([1, 8], f32, tag="osb")
    nc.scalar.activation(osb[:], o[:], AF.Identity, bias=b2sb[:, 0:1], scale=1.0)
    nc.sync.dma_start(out, osb[:])


def _prep_inputs(inputs):
    import ml_dtypes

    x = np.ascontiguousarray(np.asarray(inputs["x"], dtype=np.float32))
    batch = np.asarray(inputs["batch"]).astype(np.int64)
    Wn = np.asarray(inputs["Wn"], dtype=np.float32)
    ln_scale = np.asarray(inputs["ln_scale"], dtype=np.float32)
    ln_bias = np.asarray(inputs["ln_bias"], dtype=np.float32)
    W1 = np.asarray(inputs["W1"], dtype=np.float32)
    b1 = np.asarray(inputs["b1"], dtype=np.float32)
    W2 = np.asarray(inputs["W2"], dtype=np.float32)
    b2 = np.asarray(inputs["b2"], dtype=np.float32)
    assert np.allclose(ln_bias, 0.0), "kernel assumes ln_bias == 0"

    C = (np.eye(HID) - np.ones((HID, HID)) / HID).astype(np.float32)
    Bstar = np.eye(HID, dtype=np.float32)
    for l in range(4):
        A = np.eye(HID, dtype=np.float32) + (Wn[l, 0] + Wn[l, 1]) * 0.5
        S = (
            np.diag(ln_scale[l - 1]).astype(np.float32)
            if l > 0 else np.eye(HID, dtype=np.float32)
        )
        Bstar = Bstar @ (S @ A @ C)
    Bstar = np.ascontiguousarray(Bstar.astype(np.float32))
    W1p = np.ascontiguousarray(
        (np.diag(ln_scale[3]).astype(np.float32) @ W1).astype(np.float32)
    )

    bounds = np.searchsorted(batch, np.arange(0, 65, GPC))
    in_maps = []
    for c in range(NCORES):
        s, e = int(bounds[c]), int(bounds[c + 1])
        n = e - s
        assert n <= NPAD, f"core {c} shard {n} > NPAD {NPAD}"
        xTc = np.zeros((64, NPAD), dtype=np.float32)
        xTc[:, :n] = x[s:e].T
        mp = np.zeros((128, NBLK * GPC), dtype=np.float32)
        gb = (batch[s:e] - GPC * c).astype(np.int64)
        idx = np.arange(n)
        mp[idx % 128, (idx // 128) * GPC + gb] = 1.0
        in_maps.append(
            dict(
                xT=xTc,
                Bs=Bstar,
                Mp=np.ascontiguousarray(mp),
                W1=W1p,
                b1=np.ascontiguousarray(b1.reshape(32, 1)),
                W2=np.ascontiguousarray(W2.reshape(32, 1)),
                b2=np.ascontiguousarray(b2.reshape(1, 1)),
                ey=np.eye(8, dtype=np.float32),
            )
        )
    return in_maps


def kernel(**inputs):
    global _prog
    from concourse import bass_utils

    in_maps = _prep_inputs(inputs)
    if _prog is None:
        _prog = _build_program()
    res = bass_utils.run_bass_kernel_spmd(
        _prog, in_maps, core_ids=list(range(NCORES))
    )
    outs = [np.asarray(res.results[c]["out"]).reshape(GPC) for c in range(NCORES)]
    return np.concatenate(outs).reshape(64, 1).astype(np.float32)
